# revision 27
# baseline (speedup 1.0000x reference)
"""Trainium2 Bass kernel for a dense (length-1 sequence) Mamba block.

The reference computation reduces algebraically to:
    z   = x @ in_w                                  # (B, d_inner)
    g   = silu(z * c + b_eff)                       # per-channel scale/bias
    out = g @ out_w + out_b                         # (B, d_model)
with
    c     = conv_w[:, -1] + softplus(dt) * sum(B*C, -1) + Dp
    b_eff = (in_b * c) + conv_b
(c, b_eff are tiny per-channel vectors, computed on host.)

Strategy: data-parallel over 8 NeuronCores (batch 32768 -> 8 x 4096).
All-bf16 datapath (inputs converted on host; well within tolerance).
Per core, batch is processed in tiles of BT rows:
  x^T tiles are produced by the XBAR DMA-transpose (HBM -> SBUF) so the
  PE array spends zero cycles on transposes.
  phase M1: z^T[di, b] accumulated over d_model; Silu fused on ScalarE
            with per-partition scale/bias -> g [di, b] (bf16)
  phase M2: out[b, dm] accumulated over d_inner with g slices as the
            stationary operand and out_w tiles moving (natural output
            layout); out_b added on drain.
The kernel is PE-bound (2 x 4096x2048x4096 MACs/core); everything else
overlaps under the matmul stream.
"""

import numpy as np

import concourse.bass as bass
import concourse.tile as tile
from concourse import bacc, mybir
from concourse.bass_utils import run_bass_kernel_spmd

P = 128
B_FULL = 32768
DM = 2048
DI = 4096
N_CORES = 8
BS = B_FULL // N_CORES  # rows per core

F32 = mybir.dt.float32
BF16 = mybir.dt.bfloat16
SILU = mybir.ActivationFunctionType.Silu

import os as _os
PREFETCH_IW = _os.environ.get("K_PREFETCH", "1") == "1"
# NOTE: concurrent XBAR DMA-transposes on both HWDGE queues corrupt data
# (shared xbar state) — keep all transposes on the scalar queue.
SPLIT_PROLOGUE = _os.environ.get("K_SPLIT_PRO", "0") == "1"
IW_TAG = _os.environ.get("K_IW_TAG", "1") == "1"
WARMUP_MMS = int(_os.environ.get("K_WARMUP", "48"))

KT = DM // P            # 16 k-chunks for matmul 1
NDI = DI // P           # 32 d_inner chunks of 128
NDM = DM // 512         # 4 d_model chunks of 512
DIG = 4                 # d_inner chunks per out_w DMA batch
NDG = NDI // DIG        # 8 out_w loads per (dmc, grp)
GRP = 4                 # psum banks used by M2 accumulation


def build_nc(cfg):
    """Build the per-core Bass module. cfg: dict(BT=...)"""
    BT = cfg["BT"]
    BS_ = cfg.get("BS", BS)
    NBT = BS_ // BT         # batch tiles per core
    NB_SUB = BT // P        # 128-row subtiles per batch tile
    H = BT // 512           # moving-dim halves for matmul 1
    NGRP = NB_SUB // GRP

    nc = bacc.Bacc("TRN2", target_bir_lowering=False, debug=False,
                   num_devices=N_CORES)

    x_d = nc.dram_tensor("x16", [BS_ // BT, KT, BT, P], BF16,
                         kind="ExternalInput").ap()
    iw_d = nc.dram_tensor("iwp", [NDI, P, KT * P], BF16,
                          kind="ExternalInput").ap()
    ow_d = nc.dram_tensor("owp", [NDM, NDG, P, DIG * 512], BF16,
                          kind="ExternalInput").ap()
    c_d = nc.dram_tensor("cpb", [P, NDI], F32, kind="ExternalInput").ap()
    b_d = nc.dram_tensor("bpb", [P, NDI], F32, kind="ExternalInput").ap()
    ob_d = nc.dram_tensor("ob", [P, DM], F32, kind="ExternalInput").ap()
    out_d = nc.dram_tensor("out", [BS_, DM], F32, kind="ExternalOutput").ap()

    with tile.TileContext(nc) as tc:
        with (
            tc.tile_pool(name="const", bufs=1) as const,
            tc.tile_pool(name="xT", bufs=1) as xTp,
            tc.tile_pool(name="g", bufs=1) as gp,
            tc.tile_pool(name="iw", bufs=10) as iwp,
            tc.tile_pool(name="ow", bufs=6) as owp,
            tc.tile_pool(name="osb", bufs=2) as osbp,
            tc.tile_pool(name="psZ", bufs=3, space="PSUM") as psZ,
            tc.tile_pool(name="psO", bufs=5, space="PSUM") as psO,
        ):
            c_sb = const.tile([P, NDI], F32)
            nc.sync.dma_start(c_sb[:], c_d)
            b_sb = const.tile([P, NDI], F32)
            nc.sync.dma_start(b_sb[:], b_d)
            ob_sb = const.tile([P, DM], F32)
            nc.sync.dma_start(ob_sb[:], ob_d)

            # one tile per k-slab so RAW/WAR deps are per-slab, not whole-xT
            xT = [xTp.tile([P, BT], BF16, tag=f"xT{kt}", name=f"xT{kt}")
                  for kt in range(KT)]
            g = gp.tile([P, NDI, BT], BF16)

            def emit_xT(t, kt, eng=None):
                """XBAR DMA-transpose one [BT, 128] slab of x into xT.

                All transposes at any point in time must share one DMA
                queue: concurrent XBAR transposes on both HWDGE queues
                corrupt data (shared xbar state). Prologue transposes
                (sync) all finish before M1(0) ends; steady-state ones
                (scalar) start only after M1(t) ends — never concurrent.
                """
                (eng or nc.scalar).dma_start(
                    xT[kt][:], x_d[t, kt], transpose=True)

            def load_iw(di):
                if IW_TAG:
                    iw_t = iwp.tile([P, KT, P], BF16, tag="iw_t", name="iw_t")
                else:
                    iw_t = iwp.tile([P, KT, P], BF16, name="iw_t")
                nc.sync.dma_start(iw_t[:], iw_d[di].rearrange(
                    "p (kt m) -> p kt m", kt=KT))
                return iw_t

            def load_ow(dmc, dg):
                ow_t = owp.tile([P, DIG, 512], BF16, tag="ow_t", name="ow_t")
                nc.scalar.dma_start(
                    ow_t[:], ow_d[dmc, dg].rearrange(
                        "p (s n) -> p s n", s=DIG))
                return ow_t

            iw_pf = {}
            ow_pf = {}
            for t in range(NBT):
                if t == 0:
                    # a couple of in_w chunks ahead of the transposes on the
                    # sync queue so M1 can start as soon as slab 0 lands
                    for d in range(2):
                        iw_pf[d] = load_iw(d)
                    # prologue transposes on sync: scalar queue must stay
                    # free for the first activations (psZ turnover)
                    for kt in range(KT):
                        emit_xT(0, kt, nc.sync)
                    # warm-up matmuls: keep the PE busy through the
                    # prologue so the HAM clock-gate reaches (and holds)
                    # 8/8 before the real stream begins
                    dw = const.tile([P, P], BF16)
                    nc.vector.memset(dw[:], 0.0)
                    dm = const.tile([P, 512], BF16)
                    nc.vector.memset(dm[:], 0.0)
                    for w in range(WARMUP_MMS):
                        dps = psZ.tile([P, 512], F32, tag="zp",
                                       name=f"warm_{w}")
                        nc.tensor.matmul(dps[:], dw[:], dm[:],
                                         start=True, stop=True)

                # ---- phase M1: z^T = in_w^T @ x^T ; g = silu(z*c + b) ----
                # kick the first out_w loads of the upcoming M2 phase ahead
                # of the activation stream on the scalar queue
                for dg0 in range(2):
                    ow_pf[(0, dg0)] = load_ow(0, dg0)
                for di in range(NDI):
                    iw_t = iw_pf.pop(di, None)
                    if iw_t is None:
                        iw_t = load_iw(di)
                    zps = [psZ.tile([P, 512], F32, tag="zp", name=f"zp_{h}")
                           for h in range(H)]
                    for kt in range(KT):
                        for h in range(H):
                            nc.tensor.matmul(
                                zps[h][:],
                                iw_t[:, kt, :],
                                xT[kt][:, h * 512:(h + 1) * 512],
                                start=(kt == 0), stop=(kt == KT - 1))
                    for h in range(H):
                        nc.scalar.activation(
                            g[:, di, h * 512:(h + 1) * 512], zps[h][:], SILU,
                            bias=b_sb[:, di:di + 1], scale=c_sb[:, di:di + 1])

                # ---- phase M2: out = g^T @ out_w + out_b ----
                # x^T DMA-transposes for batch tile t+1 are spread over the
                # M2 window (2 per psum group).
                ui = 0
                for dmc in range(NDM):
                    for grp in range(NGRP):
                        ops = [psO.tile([P, 512], F32, tag="ps_o",
                                        name=f"ops_{j}")
                               for j in range(GRP)]
                        for dg in range(NDG):
                            ow_t = ow_pf.pop((dmc, dg), None)
                            if ow_t is None:
                                ow_t = load_ow(dmc, dg)
                            for s in range(DIG):
                                di = dg * DIG + s
                                for j in range(GRP):
                                    bs = grp * GRP + j
                                    nc.tensor.matmul(
                                        ops[j][:],
                                        g[:, di, bs * P:(bs + 1) * P],
                                        ow_t[:, s, :],
                                        start=(di == 0),
                                        stop=(di == NDI - 1))
                        osb = osbp.tile([P, GRP, 512], F32)
                        for j in range(GRP):
                            nc.vector.tensor_tensor(
                                osb[:, j, :], ops[j][:],
                                ob_sb[:, dmc * 512:(dmc + 1) * 512],
                                mybir.AluOpType.add)
                        r0 = t * BT + grp * GRP * P
                        nc.gpsimd.dma_start(
                            out_d[r0:r0 + GRP * P,
                                  dmc * 512:(dmc + 1) * 512].rearrange(
                                      "(s p) n -> p s n", p=P),
                            osb[:])
                        if t + 1 < NBT:
                            for _ in range(2):
                                if ui < KT:
                                    emit_xT(t + 1, ui)
                                    ui += 1
                            # prefetch next tile's first in_w chunks so the
                            # M1 phase never waits on the sync DMA queue
                            pf = dmc * NGRP + grp
                            if pf < 8 and PREFETCH_IW:
                                iw_pf[pf] = load_iw(pf)
    nc.compile()
    return nc


_NC_CACHE = {}


def _get_nc(key):
    if key not in _NC_CACHE:
        cfg = dict(BT=key[0])
        _NC_CACHE[key] = build_nc(cfg)
    return _NC_CACHE[key]


CONFIG = (1024,)


def _softplus(v):
    return np.logaddexp(0.0, v)


def prep_inputs(x, in_w, in_b, conv_w, conv_b, A_log, B, C, Dp, dt,
                out_w, out_b):
    """Host-side prep shared by kernel() and the test harness."""
    import ml_dtypes
    bf16 = ml_dtypes.bfloat16

    bt = CONFIG[0]
    x16 = np.asarray(x, np.float32).astype(bf16)
    # contiguous [BT, 128] slabs so each XBAR DMA-transpose reads DRAM
    # sequentially: [BS, DM] -> [NBT, KT, BT, P]
    x16 = x16.reshape(N_CORES, BS // bt, bt, KT, P).transpose(0, 1, 3, 2, 4)

    # in_w [DM, DI] -> iwp [NDI, P(dm-in-chunk), KT*P(di-in-chunk)]
    iw = np.asarray(in_w, np.float32).astype(bf16)
    iwp = np.ascontiguousarray(
        iw.reshape(KT, P, NDI, P).transpose(2, 1, 0, 3).reshape(
            NDI, P, KT * P))

    # out_w [DI, DM] -> owp [NDM, NDG, P(di-in-chunk), DIG*512]
    ow = np.asarray(out_w, np.float32).astype(bf16)
    owp = np.ascontiguousarray(
        ow.reshape(NDG, DIG, P, NDM, 512).transpose(3, 0, 2, 1, 4).reshape(
            NDM, NDG, P, DIG * 512))

    # host precompute of the per-channel SSM/conv collapse
    c = (np.asarray(conv_w, np.float32)[:, -1]
         + _softplus(np.asarray(dt, np.float32))
         * np.sum(np.asarray(B, np.float32) * np.asarray(C, np.float32), -1)
         + np.asarray(Dp, np.float32))
    b_eff = np.asarray(in_b, np.float32) * c + np.asarray(conv_b, np.float32)

    c_pb = np.ascontiguousarray(c.reshape(NDI, P).T)
    b_pb = np.ascontiguousarray(b_eff.reshape(NDI, P).T)
    ob_rep = np.ascontiguousarray(
        np.broadcast_to(np.asarray(out_b, np.float32), (P, DM)))

    in_maps = []
    for i in range(N_CORES):
        in_maps.append({
            "x16": np.ascontiguousarray(x16[i]),
            "iwp": iwp,
            "owp": owp,
            "cpb": c_pb,
            "bpb": b_pb,
            "ob": ob_rep,
        })
    return in_maps


def kernel(x, in_w, in_b, conv_w, conv_b, A_log, B, C, Dp, dt, out_w, out_b):
    in_maps = prep_inputs(x, in_w, in_b, conv_w, conv_b, A_log, B, C, Dp,
                          dt, out_w, out_b)
    nc = _get_nc(CONFIG)
    out = np.empty((B_FULL, DM), dtype=np.float32)
    try:
        res = run_bass_kernel_spmd(nc, in_maps, core_ids=list(range(N_CORES)))
        for i in range(N_CORES):
            out[i * BS:(i + 1) * BS] = res.results[i]["out"]
    except Exception:
        # The accelerator occasionally hits a transient unrecoverable fault
        # that poisons this process's PJRT client; a fresh process recovers.
        # Retry the device execution in a subprocess.
        _run_in_subprocess(in_maps, out)
    return out


def _run_in_subprocess(in_maps, out):
    import pickle
    import subprocess
    import sys
    import tempfile

    with tempfile.TemporaryDirectory() as td:
        in_path = f"{td}/in.pkl"
        out_path = f"{td}/out.npy"
        with open(in_path, "wb") as f:
            pickle.dump({"config": CONFIG, "in_maps": in_maps}, f,
                        protocol=pickle.HIGHEST_PROTOCOL)
        for attempt in range(3):
            r = subprocess.run(
                [sys.executable, __file__, "--worker", in_path, out_path],
                capture_output=True)
            if r.returncode == 0:
                break
            if attempt == 2:
                raise RuntimeError(
                    f"device worker failed 3x: {r.stderr[-2000:]!r}")
        out[:] = np.load(out_path)


def _worker_main(in_path, out_path):
    import pickle
    with open(in_path, "rb") as f:
        job = pickle.load(f)
    nc = _get_nc(tuple(job["config"]))
    res = run_bass_kernel_spmd(nc, job["in_maps"],
                               core_ids=list(range(N_CORES)))
    out = np.empty((B_FULL, DM), dtype=np.float32)
    for i in range(N_CORES):
        out[i * BS:(i + 1) * BS] = res.results[i]["out"]
    np.save(out_path, out)


if __name__ == "__main__":
    import sys as _sys
    if len(_sys.argv) == 4 and _sys.argv[1] == "--worker":
        _worker_main(_sys.argv[2], _sys.argv[3])


# revision 38
# speedup vs baseline: 1.0179x; 1.0179x over previous
"""Trainium2 Bass kernel for a dense (length-1 sequence) Mamba block.

The reference computation reduces algebraically to:
    z   = x @ in_w                                  # (B, d_inner)
    g   = silu(z * c + b_eff)                       # per-channel scale/bias
    out = g @ out_w + out_b                         # (B, d_model)
with
    c     = conv_w[:, -1] + softplus(dt) * sum(B*C, -1) + Dp
    b_eff = (in_b * c) + conv_b
(c, b_eff are tiny per-channel vectors, computed on host.)

Strategy: data-parallel over 8 NeuronCores (batch 32768 -> 8 x 4096).
All-bf16 datapath (inputs converted on host; well within tolerance).
Per core, batch is processed in tiles of BT rows:
  x^T tiles are produced by the XBAR DMA-transpose (HBM -> SBUF) so the
  PE array spends zero cycles on transposes.
  phase M1: z^T[di, b] accumulated over d_model; Silu fused on ScalarE
            with per-partition scale/bias -> g [di, b] (bf16)
  phase M2: out[b, dm] accumulated over d_inner with g slices as the
            stationary operand and out_w tiles moving (natural output
            layout); out_b added on drain.
The kernel is PE-bound (2 x 4096x2048x4096 MACs/core); everything else
overlaps under the matmul stream.
"""

import numpy as np

import concourse.bass as bass
import concourse.tile as tile
from concourse import bacc, mybir
from concourse.bass_utils import run_bass_kernel_spmd

P = 128
B_FULL = 32768
DM = 2048
DI = 4096
N_CORES = 8
BS = B_FULL // N_CORES  # rows per core

F32 = mybir.dt.float32
BF16 = mybir.dt.bfloat16
SILU = mybir.ActivationFunctionType.Silu

import os as _os
PREFETCH_IW = _os.environ.get("K_PREFETCH", "1") == "1"
# NOTE: concurrent XBAR DMA-transposes on both HWDGE queues corrupt data
# (shared xbar state) — keep all transposes on the scalar queue.
SPLIT_PROLOGUE = _os.environ.get("K_SPLIT_PRO", "0") == "1"
IW_TAG = _os.environ.get("K_IW_TAG", "1") == "1"
WARMUP_MMS = int(_os.environ.get("K_WARMUP", "48"))

KT = DM // P            # 16 k-chunks for matmul 1
NDI = DI // P           # 32 d_inner chunks of 128
NDM = DM // 512         # 4 d_model chunks of 512
DIG = 4                 # d_inner chunks per out_w DMA batch
NDG = NDI // DIG        # 8 out_w loads per (dmc, grp)
GRP = 4                 # psum banks used by M2 accumulation


def build_nc(cfg):
    """Build the per-core Bass module. cfg: dict(BT=...)"""
    BT = cfg["BT"]
    BS_ = cfg.get("BS", BS)
    NBT = BS_ // BT         # batch tiles per core
    NB_SUB = BT // P        # 128-row subtiles per batch tile
    H = BT // 512           # moving-dim halves for matmul 1
    NGRP = NB_SUB // GRP

    nc = bacc.Bacc("TRN2", target_bir_lowering=False, debug=False,
                   num_devices=N_CORES)

    x_d = nc.dram_tensor("x16", [BS_ // BT, KT, BT, P], BF16,
                         kind="ExternalInput").ap()
    iw_d = nc.dram_tensor("iwp", [NDI, P, KT * P], BF16,
                          kind="ExternalInput").ap()
    ow_d = nc.dram_tensor("owp", [NDM, NDG, P, DIG * 512], BF16,
                          kind="ExternalInput").ap()
    c_d = nc.dram_tensor("cpb", [P, NDI], F32, kind="ExternalInput").ap()
    b_d = nc.dram_tensor("bpb", [P, NDI], F32, kind="ExternalInput").ap()
    ob_d = nc.dram_tensor("ob", [P, DM], F32, kind="ExternalInput").ap()
    out_d = nc.dram_tensor("out", [BS_, DM], F32, kind="ExternalOutput").ap()

    with tile.TileContext(nc) as tc:
        with (
            tc.tile_pool(name="const", bufs=1) as const,
            tc.tile_pool(name="xT", bufs=1) as xTp,
            tc.tile_pool(name="g", bufs=1) as gp,
            tc.tile_pool(name="iw", bufs=6) as iwp,
            tc.tile_pool(name="ow", bufs=8) as owp,
            tc.tile_pool(name="osb", bufs=2) as osbp,
            tc.tile_pool(name="psZ", bufs=3, space="PSUM") as psZ,
            tc.tile_pool(name="psO", bufs=5, space="PSUM") as psO,
        ):
            c_sb = const.tile([P, NDI], F32)
            nc.sync.dma_start(c_sb[:], c_d)
            b_sb = const.tile([P, NDI], F32)
            nc.sync.dma_start(b_sb[:], b_d)
            ob_sb = const.tile([P, DM], F32)
            nc.sync.dma_start(ob_sb[:], ob_d)

            # one tile per k-slab so RAW/WAR deps are per-slab. Single
            # buffer per slab: the WAR on M1(t)'s reads is what defers
            # tile t+1's transposes into the M2(t) window — the scheduler
            # orders ready instructions greedily, so only data deps can
            # keep the scalar-queue transposes clear of the sync-queue
            # prologue (concurrent XBAR transposes on two queues corrupt).
            g = gp.tile([P, NDI, BT], BF16)
            xT = {}

            def emit_xT(t, kt, eng=None):
                """XBAR DMA-transpose one [BT, 128] slab of x into xT."""
                slab = xTp.tile([P, BT], BF16, tag=f"xT{kt}",
                                name=f"xT{kt}_{t}")
                xT[(t, kt)] = slab
                (eng or nc.scalar).dma_start(
                    slab[:], x_d[t, kt], transpose=True)

            def load_iw(di):
                if IW_TAG:
                    iw_t = iwp.tile([P, KT, P], BF16, tag="iw_t", name="iw_t")
                else:
                    iw_t = iwp.tile([P, KT, P], BF16, name="iw_t")
                nc.sync.dma_start(iw_t[:], iw_d[di].rearrange(
                    "p (kt m) -> p kt m", kt=KT))
                return iw_t

            def load_ow(dmc, dg):
                ow_t = owp.tile([P, DIG, 512], BF16, tag="ow_t", name="ow_t")
                nc.sync.dma_start(
                    ow_t[:], ow_d[dmc, dg].rearrange(
                        "p (s n) -> p s n", s=DIG))
                return ow_t

            iw_pf = {}
            for t in range(NBT):
                if t == 0:
                    # a couple of in_w chunks ahead of the transposes on the
                    # sync queue so M1 can start as soon as slab 0 lands
                    for d in range(2):
                        iw_pf[d] = load_iw(d)
                    # prologue transposes on sync: scalar queue must stay
                    # free for the first activations (psZ turnover)
                    for kt in range(KT):
                        emit_xT(0, kt, nc.sync)
                    # warm-up matmuls: keep the PE busy through the
                    # prologue so the HAM clock-gate reaches (and holds)
                    # 8/8 before the real stream begins
                    dw = const.tile([P, P], BF16)
                    nc.vector.memset(dw[:], 0.0)
                    dm = const.tile([P, 512], BF16)
                    nc.vector.memset(dm[:], 0.0)
                    for w in range(WARMUP_MMS):
                        dps = psZ.tile([P, 512], F32, tag="zp",
                                       name=f"warm_{w}")
                        nc.tensor.matmul(dps[:], dw[:], dm[:],
                                         start=True, stop=True)

                # ---- phase M1: z^T = in_w^T @ x^T ; g = silu(z*c + b) ----
                for di in range(NDI):
                    iw_t = iw_pf.pop(di, None)
                    if iw_t is None:
                        iw_t = load_iw(di)
                    zps = [psZ.tile([P, 512], F32, tag="zp", name=f"zp_{h}")
                           for h in range(H)]
                    for kt in range(KT):
                        for h in range(H):
                            nc.tensor.matmul(
                                zps[h][:],
                                iw_t[:, kt, :],
                                xT[(t, kt)][:, h * 512:(h + 1) * 512],
                                start=(kt == 0), stop=(kt == KT - 1))
                    for h in range(H):
                        nc.scalar.activation(
                            g[:, di, h * 512:(h + 1) * 512], zps[h][:], SILU,
                            bias=b_sb[:, di:di + 1], scale=c_sb[:, di:di + 1])
                # ---- phase M2: out = g^T @ out_w + out_b ----
                ui = 0
                for dmc in range(NDM):
                    for grp in range(NGRP):
                        ops = [psO.tile([P, 512], F32, tag="ps_o",
                                        name=f"ops_{j}")
                               for j in range(GRP)]
                        for dg in range(NDG):
                            ow_t = load_ow(dmc, dg)
                            for s in range(DIG):
                                di = dg * DIG + s
                                for j in range(GRP):
                                    bs = grp * GRP + j
                                    nc.tensor.matmul(
                                        ops[j][:],
                                        g[:, di, bs * P:(bs + 1) * P],
                                        ow_t[:, s, :],
                                        start=(di == 0),
                                        stop=(di == NDI - 1))
                        osb = osbp.tile([P, GRP, 512], F32)
                        for j in range(GRP):
                            nc.vector.tensor_tensor(
                                osb[:, j, :], ops[j][:],
                                ob_sb[:, dmc * 512:(dmc + 1) * 512],
                                mybir.AluOpType.add)
                        r0 = t * BT + grp * GRP * P
                        nc.gpsimd.dma_start(
                            out_d[r0:r0 + GRP * P,
                                  dmc * 512:(dmc + 1) * 512].rearrange(
                                      "(s p) n -> p s n", p=P),
                            osb[:])
                        if t + 1 < NBT:
                            for _ in range(2):
                                if ui < KT:
                                    emit_xT(t + 1, ui)
                                    ui += 1
                            # prefetch next tile's first in_w chunks in the
                            # second half of M2, after the transpose burst
                            pf = dmc * NGRP + grp
                            if 4 <= pf < 8 and PREFETCH_IW:
                                iw_pf[pf - 4] = load_iw(pf - 4)
    nc.compile()
    return nc


_NC_CACHE = {}


def _get_nc(key):
    if key not in _NC_CACHE:
        cfg = dict(BT=key[0])
        _NC_CACHE[key] = build_nc(cfg)
    return _NC_CACHE[key]


CONFIG = (1024,)


def _softplus(v):
    return np.logaddexp(0.0, v)


def prep_inputs(x, in_w, in_b, conv_w, conv_b, A_log, B, C, Dp, dt,
                out_w, out_b):
    """Host-side prep shared by kernel() and the test harness."""
    import ml_dtypes
    bf16 = ml_dtypes.bfloat16

    bt = CONFIG[0]
    x16 = np.asarray(x, np.float32).astype(bf16)
    # contiguous [BT, 128] slabs so each XBAR DMA-transpose reads DRAM
    # sequentially: [BS, DM] -> [NBT, KT, BT, P]
    x16 = x16.reshape(N_CORES, BS // bt, bt, KT, P).transpose(0, 1, 3, 2, 4)

    # in_w [DM, DI] -> iwp [NDI, P(dm-in-chunk), KT*P(di-in-chunk)]
    iw = np.asarray(in_w, np.float32).astype(bf16)
    iwp = np.ascontiguousarray(
        iw.reshape(KT, P, NDI, P).transpose(2, 1, 0, 3).reshape(
            NDI, P, KT * P))

    # out_w [DI, DM] -> owp [NDM, NDG, P(di-in-chunk), DIG*512]
    ow = np.asarray(out_w, np.float32).astype(bf16)
    owp = np.ascontiguousarray(
        ow.reshape(NDG, DIG, P, NDM, 512).transpose(3, 0, 2, 1, 4).reshape(
            NDM, NDG, P, DIG * 512))

    # host precompute of the per-channel SSM/conv collapse
    c = (np.asarray(conv_w, np.float32)[:, -1]
         + _softplus(np.asarray(dt, np.float32))
         * np.sum(np.asarray(B, np.float32) * np.asarray(C, np.float32), -1)
         + np.asarray(Dp, np.float32))
    b_eff = np.asarray(in_b, np.float32) * c + np.asarray(conv_b, np.float32)

    c_pb = np.ascontiguousarray(c.reshape(NDI, P).T)
    b_pb = np.ascontiguousarray(b_eff.reshape(NDI, P).T)
    ob_rep = np.ascontiguousarray(
        np.broadcast_to(np.asarray(out_b, np.float32), (P, DM)))

    in_maps = []
    for i in range(N_CORES):
        in_maps.append({
            "x16": np.ascontiguousarray(x16[i]),
            "iwp": iwp,
            "owp": owp,
            "cpb": c_pb,
            "bpb": b_pb,
            "ob": ob_rep,
        })
    return in_maps


def kernel(x, in_w, in_b, conv_w, conv_b, A_log, B, C, Dp, dt, out_w, out_b):
    in_maps = prep_inputs(x, in_w, in_b, conv_w, conv_b, A_log, B, C, Dp,
                          dt, out_w, out_b)
    nc = _get_nc(CONFIG)
    out = np.empty((B_FULL, DM), dtype=np.float32)
    try:
        res = run_bass_kernel_spmd(nc, in_maps, core_ids=list(range(N_CORES)))
        for i in range(N_CORES):
            out[i * BS:(i + 1) * BS] = res.results[i]["out"]
    except Exception:
        # The accelerator occasionally hits a transient unrecoverable fault
        # that poisons this process's PJRT client; a fresh process recovers.
        # Retry the device execution in a subprocess.
        _run_in_subprocess(in_maps, out)
    return out


def _run_in_subprocess(in_maps, out):
    import pickle
    import subprocess
    import sys
    import tempfile

    with tempfile.TemporaryDirectory() as td:
        in_path = f"{td}/in.pkl"
        out_path = f"{td}/out.npy"
        with open(in_path, "wb") as f:
            pickle.dump({"config": CONFIG, "in_maps": in_maps}, f,
                        protocol=pickle.HIGHEST_PROTOCOL)
        for attempt in range(3):
            r = subprocess.run(
                [sys.executable, __file__, "--worker", in_path, out_path],
                capture_output=True)
            if r.returncode == 0:
                break
            if attempt == 2:
                raise RuntimeError(
                    f"device worker failed 3x: {r.stderr[-2000:]!r}")
        out[:] = np.load(out_path)


def _worker_main(in_path, out_path):
    import pickle
    with open(in_path, "rb") as f:
        job = pickle.load(f)
    nc = _get_nc(tuple(job["config"]))
    res = run_bass_kernel_spmd(nc, job["in_maps"],
                               core_ids=list(range(N_CORES)))
    out = np.empty((B_FULL, DM), dtype=np.float32)
    for i in range(N_CORES):
        out[i * BS:(i + 1) * BS] = res.results[i]["out"]
    np.save(out_path, out)


if __name__ == "__main__":
    import sys as _sys
    if len(_sys.argv) == 4 and _sys.argv[1] == "--worker":
        _worker_main(_sys.argv[2], _sys.argv[3])


# revision 39
# speedup vs baseline: 1.0318x; 1.0137x over previous
"""Trainium2 Bass kernel for a dense (length-1 sequence) Mamba block.

The reference computation reduces algebraically to:
    z   = x @ in_w                                  # (B, d_inner)
    g   = silu(z * c + b_eff)                       # per-channel scale/bias
    out = g @ out_w + out_b                         # (B, d_model)
with
    c     = conv_w[:, -1] + softplus(dt) * sum(B*C, -1) + Dp
    b_eff = (in_b * c) + conv_b
(c, b_eff are tiny per-channel vectors, computed on host.)

Strategy: data-parallel over 8 NeuronCores (batch 32768 -> 8 x 4096).
All-bf16 datapath (inputs converted on host; well within tolerance).
Per core, batch is processed in tiles of BT rows:
  x^T tiles are produced by the XBAR DMA-transpose (HBM -> SBUF) so the
  PE array spends zero cycles on transposes.
  phase M1: z^T[di, b] accumulated over d_model; Silu fused on ScalarE
            with per-partition scale/bias -> g [di, b] (bf16)
  phase M2: out[b, dm] accumulated over d_inner with g slices as the
            stationary operand and out_w tiles moving (natural output
            layout); out_b added on drain.
The kernel is PE-bound (2 x 4096x2048x4096 MACs/core); everything else
overlaps under the matmul stream.
"""

import numpy as np

import concourse.bass as bass
import concourse.tile as tile
from concourse import bacc, mybir
from concourse.bass_utils import run_bass_kernel_spmd

P = 128
B_FULL = 32768
DM = 2048
DI = 4096
N_CORES = 8
BS = B_FULL // N_CORES  # rows per core

F32 = mybir.dt.float32
BF16 = mybir.dt.bfloat16
SILU = mybir.ActivationFunctionType.Silu

import os as _os
PREFETCH_IW = _os.environ.get("K_PREFETCH", "1") == "1"
# NOTE: concurrent XBAR DMA-transposes on both HWDGE queues corrupt data
# (shared xbar state) — keep all transposes on the scalar queue.
SPLIT_PROLOGUE = _os.environ.get("K_SPLIT_PRO", "0") == "1"
IW_TAG = _os.environ.get("K_IW_TAG", "1") == "1"
WARMUP_MMS = int(_os.environ.get("K_WARMUP", "48"))

KT = DM // P            # 16 k-chunks for matmul 1
NDI = DI // P           # 32 d_inner chunks of 128
NDM = DM // 512         # 4 d_model chunks of 512
DIG = 4                 # d_inner chunks per out_w DMA batch
NDG = NDI // DIG        # 8 out_w loads per (dmc, grp)
GRP = 4                 # psum banks used by M2 accumulation


def build_nc(cfg):
    """Build the per-core Bass module. cfg: dict(BT=...)"""
    BT = cfg["BT"]
    BS_ = cfg.get("BS", BS)
    NBT = BS_ // BT         # batch tiles per core
    NB_SUB = BT // P        # 128-row subtiles per batch tile
    H = BT // 512           # moving-dim halves for matmul 1
    NGRP = NB_SUB // GRP

    nc = bacc.Bacc("TRN2", target_bir_lowering=False, debug=False,
                   num_devices=N_CORES)

    x_d = nc.dram_tensor("x16", [BS_ // BT, KT, BT, P], BF16,
                         kind="ExternalInput").ap()
    iw_d = nc.dram_tensor("iwp", [NDI, P, KT * P], BF16,
                          kind="ExternalInput").ap()
    ow_d = nc.dram_tensor("owp", [NDM, NDG, P, DIG * 512], BF16,
                          kind="ExternalInput").ap()
    c_d = nc.dram_tensor("cpb", [P, NDI], F32, kind="ExternalInput").ap()
    b_d = nc.dram_tensor("bpb", [P, NDI], F32, kind="ExternalInput").ap()
    ob_d = nc.dram_tensor("ob", [P, DM], F32, kind="ExternalInput").ap()
    out_d = nc.dram_tensor("out", [BS_, DM], F32, kind="ExternalOutput").ap()

    with tile.TileContext(nc) as tc:
        with (
            tc.tile_pool(name="const", bufs=1) as const,
            tc.tile_pool(name="xT", bufs=1) as xTp,
            tc.tile_pool(name="g", bufs=1) as gp,
            tc.tile_pool(name="iw", bufs=6) as iwp,
            tc.tile_pool(name="ow", bufs=12) as owp,
            tc.tile_pool(name="osb", bufs=2) as osbp,
            tc.tile_pool(name="psZ", bufs=3, space="PSUM") as psZ,
            tc.tile_pool(name="psO", bufs=5, space="PSUM") as psO,
        ):
            c_sb = const.tile([P, NDI], F32)
            nc.sync.dma_start(c_sb[:], c_d)
            b_sb = const.tile([P, NDI], F32)
            nc.sync.dma_start(b_sb[:], b_d)
            ob_sb = const.tile([P, DM], F32)
            nc.sync.dma_start(ob_sb[:], ob_d)

            # one tile per k-slab so RAW/WAR deps are per-slab. Single
            # buffer per slab: the WAR on M1(t)'s reads is what defers
            # tile t+1's transposes into the M2(t) window — the scheduler
            # orders ready instructions greedily, so only data deps can
            # keep the scalar-queue transposes clear of the sync-queue
            # prologue (concurrent XBAR transposes on two queues corrupt).
            g = gp.tile([P, NDI, BT], BF16)
            xT = {}

            def emit_xT(t, kt, eng=None):
                """XBAR DMA-transpose one [BT, 128] slab of x into xT."""
                slab = xTp.tile([P, BT], BF16, tag=f"xT{kt}",
                                name=f"xT{kt}_{t}")
                xT[(t, kt)] = slab
                (eng or nc.scalar).dma_start(
                    slab[:], x_d[t, kt], transpose=True)

            def load_iw(di):
                if IW_TAG:
                    iw_t = iwp.tile([P, KT, P], BF16, tag="iw_t", name="iw_t")
                else:
                    iw_t = iwp.tile([P, KT, P], BF16, name="iw_t")
                nc.sync.dma_start(iw_t[:], iw_d[di].rearrange(
                    "p (kt m) -> p kt m", kt=KT))
                return iw_t

            def load_ow(dmc, dg):
                ow_t = owp.tile([P, DIG, 512], BF16, tag="ow_t", name="ow_t")
                nc.sync.dma_start(
                    ow_t[:], ow_d[dmc, dg].rearrange(
                        "p (s n) -> p s n", s=DIG))
                return ow_t

            iw_pf = {}
            for t in range(NBT):
                if t == 0:
                    # a couple of in_w chunks ahead of the transposes on the
                    # sync queue so M1 can start as soon as slab 0 lands
                    for d in range(2):
                        iw_pf[d] = load_iw(d)
                    # prologue transposes on sync: scalar queue must stay
                    # free for the first activations (psZ turnover)
                    for kt in range(KT):
                        emit_xT(0, kt, nc.sync)
                    # warm-up matmuls: keep the PE busy through the
                    # prologue so the HAM clock-gate reaches (and holds)
                    # 8/8 before the real stream begins
                    dw = const.tile([P, P], BF16)
                    nc.vector.memset(dw[:], 0.0)
                    dm = const.tile([P, 512], BF16)
                    nc.vector.memset(dm[:], 0.0)
                    for w in range(WARMUP_MMS):
                        dps = psZ.tile([P, 512], F32, tag="zp",
                                       name=f"warm_{w}")
                        nc.tensor.matmul(dps[:], dw[:], dm[:],
                                         start=True, stop=True)

                # ---- phase M1: z^T = in_w^T @ x^T ; g = silu(z*c + b) ----
                for di in range(NDI):
                    iw_t = iw_pf.pop(di, None)
                    if iw_t is None:
                        iw_t = load_iw(di)
                    zps = [psZ.tile([P, 512], F32, tag="zp", name=f"zp_{h}")
                           for h in range(H)]
                    for kt in range(KT):
                        for h in range(H):
                            nc.tensor.matmul(
                                zps[h][:],
                                iw_t[:, kt, :],
                                xT[(t, kt)][:, h * 512:(h + 1) * 512],
                                start=(kt == 0), stop=(kt == KT - 1))
                    for h in range(H):
                        nc.scalar.activation(
                            g[:, di, h * 512:(h + 1) * 512], zps[h][:], SILU,
                            bias=b_sb[:, di:di + 1], scale=c_sb[:, di:di + 1])
                # ---- phase M2: out = g^T @ out_w + out_b ----
                ui = 0
                for dmc in range(NDM):
                    for grp in range(NGRP):
                        ops = [psO.tile([P, 512], F32, tag="ps_o",
                                        name=f"ops_{j}")
                               for j in range(GRP)]
                        for dg in range(NDG):
                            ow_t = load_ow(dmc, dg)
                            for s in range(DIG):
                                di = dg * DIG + s
                                for j in range(GRP):
                                    bs = grp * GRP + j
                                    nc.tensor.matmul(
                                        ops[j][:],
                                        g[:, di, bs * P:(bs + 1) * P],
                                        ow_t[:, s, :],
                                        start=(di == 0),
                                        stop=(di == NDI - 1))
                        osb = osbp.tile([P, GRP, 512], F32)
                        for j in range(GRP):
                            nc.vector.tensor_tensor(
                                osb[:, j, :], ops[j][:],
                                ob_sb[:, dmc * 512:(dmc + 1) * 512],
                                mybir.AluOpType.add)
                        r0 = t * BT + grp * GRP * P
                        nc.gpsimd.dma_start(
                            out_d[r0:r0 + GRP * P,
                                  dmc * 512:(dmc + 1) * 512].rearrange(
                                      "(s p) n -> p s n", p=P),
                            osb[:])
                        if t + 1 < NBT:
                            for _ in range(2):
                                if ui < KT:
                                    emit_xT(t + 1, ui)
                                    ui += 1
                            # prefetch next tile's first in_w chunks in the
                            # second half of M2, after the transpose burst
                            pf = dmc * NGRP + grp
                            if 4 <= pf < 8 and PREFETCH_IW:
                                iw_pf[pf - 4] = load_iw(pf - 4)
    nc.compile()
    return nc


_NC_CACHE = {}


def _get_nc(key):
    if key not in _NC_CACHE:
        cfg = dict(BT=key[0])
        _NC_CACHE[key] = build_nc(cfg)
    return _NC_CACHE[key]


CONFIG = (1024,)


def _softplus(v):
    return np.logaddexp(0.0, v)


def prep_inputs(x, in_w, in_b, conv_w, conv_b, A_log, B, C, Dp, dt,
                out_w, out_b):
    """Host-side prep shared by kernel() and the test harness."""
    import ml_dtypes
    bf16 = ml_dtypes.bfloat16

    bt = CONFIG[0]
    x16 = np.asarray(x, np.float32).astype(bf16)
    # contiguous [BT, 128] slabs so each XBAR DMA-transpose reads DRAM
    # sequentially: [BS, DM] -> [NBT, KT, BT, P]
    x16 = x16.reshape(N_CORES, BS // bt, bt, KT, P).transpose(0, 1, 3, 2, 4)

    # in_w [DM, DI] -> iwp [NDI, P(dm-in-chunk), KT*P(di-in-chunk)]
    iw = np.asarray(in_w, np.float32).astype(bf16)
    iwp = np.ascontiguousarray(
        iw.reshape(KT, P, NDI, P).transpose(2, 1, 0, 3).reshape(
            NDI, P, KT * P))

    # out_w [DI, DM] -> owp [NDM, NDG, P(di-in-chunk), DIG*512]
    ow = np.asarray(out_w, np.float32).astype(bf16)
    owp = np.ascontiguousarray(
        ow.reshape(NDG, DIG, P, NDM, 512).transpose(3, 0, 2, 1, 4).reshape(
            NDM, NDG, P, DIG * 512))

    # host precompute of the per-channel SSM/conv collapse
    c = (np.asarray(conv_w, np.float32)[:, -1]
         + _softplus(np.asarray(dt, np.float32))
         * np.sum(np.asarray(B, np.float32) * np.asarray(C, np.float32), -1)
         + np.asarray(Dp, np.float32))
    b_eff = np.asarray(in_b, np.float32) * c + np.asarray(conv_b, np.float32)

    c_pb = np.ascontiguousarray(c.reshape(NDI, P).T)
    b_pb = np.ascontiguousarray(b_eff.reshape(NDI, P).T)
    ob_rep = np.ascontiguousarray(
        np.broadcast_to(np.asarray(out_b, np.float32), (P, DM)))

    in_maps = []
    for i in range(N_CORES):
        in_maps.append({
            "x16": np.ascontiguousarray(x16[i]),
            "iwp": iwp,
            "owp": owp,
            "cpb": c_pb,
            "bpb": b_pb,
            "ob": ob_rep,
        })
    return in_maps


def kernel(x, in_w, in_b, conv_w, conv_b, A_log, B, C, Dp, dt, out_w, out_b):
    in_maps = prep_inputs(x, in_w, in_b, conv_w, conv_b, A_log, B, C, Dp,
                          dt, out_w, out_b)
    nc = _get_nc(CONFIG)
    out = np.empty((B_FULL, DM), dtype=np.float32)
    try:
        res = run_bass_kernel_spmd(nc, in_maps, core_ids=list(range(N_CORES)))
        for i in range(N_CORES):
            out[i * BS:(i + 1) * BS] = res.results[i]["out"]
    except Exception:
        # The accelerator occasionally hits a transient unrecoverable fault
        # that poisons this process's PJRT client; a fresh process recovers.
        # Retry the device execution in a subprocess.
        _run_in_subprocess(in_maps, out)
    return out


def _run_in_subprocess(in_maps, out):
    import pickle
    import subprocess
    import sys
    import tempfile

    with tempfile.TemporaryDirectory() as td:
        in_path = f"{td}/in.pkl"
        out_path = f"{td}/out.npy"
        with open(in_path, "wb") as f:
            pickle.dump({"config": CONFIG, "in_maps": in_maps}, f,
                        protocol=pickle.HIGHEST_PROTOCOL)
        for attempt in range(3):
            r = subprocess.run(
                [sys.executable, __file__, "--worker", in_path, out_path],
                capture_output=True)
            if r.returncode == 0:
                break
            if attempt == 2:
                raise RuntimeError(
                    f"device worker failed 3x: {r.stderr[-2000:]!r}")
        out[:] = np.load(out_path)


def _worker_main(in_path, out_path):
    import pickle
    with open(in_path, "rb") as f:
        job = pickle.load(f)
    nc = _get_nc(tuple(job["config"]))
    res = run_bass_kernel_spmd(nc, job["in_maps"],
                               core_ids=list(range(N_CORES)))
    out = np.empty((B_FULL, DM), dtype=np.float32)
    for i in range(N_CORES):
        out[i * BS:(i + 1) * BS] = res.results[i]["out"]
    np.save(out_path, out)


if __name__ == "__main__":
    import sys as _sys
    if len(_sys.argv) == 4 and _sys.argv[1] == "--worker":
        _worker_main(_sys.argv[2], _sys.argv[3])


# revision 41
# speedup vs baseline: 1.0362x; 1.0043x over previous
"""Trainium2 Bass kernel for a dense (length-1 sequence) Mamba block.

The reference computation reduces algebraically to:
    z   = x @ in_w                                  # (B, d_inner)
    g   = silu(z * c + b_eff)                       # per-channel scale/bias
    out = g @ out_w + out_b                         # (B, d_model)
with
    c     = conv_w[:, -1] + softplus(dt) * sum(B*C, -1) + Dp
    b_eff = (in_b * c) + conv_b
(c, b_eff are tiny per-channel vectors, computed on host.)

Strategy: data-parallel over 8 NeuronCores (batch 32768 -> 8 x 4096).
All-bf16 datapath (inputs converted on host; well within tolerance).
Per core, batch is processed in tiles of BT rows:
  x^T tiles are produced by the XBAR DMA-transpose (HBM -> SBUF) so the
  PE array spends zero cycles on transposes.
  phase M1: z^T[di, b] accumulated over d_model; Silu fused on ScalarE
            with per-partition scale/bias -> g [di, b] (bf16)
  phase M2: out[b, dm] accumulated over d_inner with g slices as the
            stationary operand and out_w tiles moving (natural output
            layout); out_b added on drain.
The kernel is PE-bound (2 x 4096x2048x4096 MACs/core); everything else
overlaps under the matmul stream.
"""

import numpy as np

import concourse.bass as bass
import concourse.tile as tile
from concourse import bacc, mybir
from concourse.bass_utils import run_bass_kernel_spmd

P = 128
B_FULL = 32768
DM = 2048
DI = 4096
N_CORES = 8
BS = B_FULL // N_CORES  # rows per core

F32 = mybir.dt.float32
BF16 = mybir.dt.bfloat16
SILU = mybir.ActivationFunctionType.Silu

import os as _os
PREFETCH_IW = _os.environ.get("K_PREFETCH", "1") == "1"
# NOTE: concurrent XBAR DMA-transposes on both HWDGE queues corrupt data
# (shared xbar state) — keep all transposes on the scalar queue.
SPLIT_PROLOGUE = _os.environ.get("K_SPLIT_PRO", "0") == "1"
IW_TAG = _os.environ.get("K_IW_TAG", "1") == "1"
WARMUP_MMS = int(_os.environ.get("K_WARMUP", "48"))

KT = DM // P            # 16 k-chunks for matmul 1
NDI = DI // P           # 32 d_inner chunks of 128
NDM = DM // 512         # 4 d_model chunks of 512
DIG = 4                 # d_inner chunks per out_w DMA batch
NDG = NDI // DIG        # 8 out_w loads per (dmc, grp)
GRP = 4                 # psum banks used by M2 accumulation


def build_nc(cfg):
    """Build the per-core Bass module. cfg: dict(BT=...)"""
    BT = cfg["BT"]
    BS_ = cfg.get("BS", BS)
    NBT = BS_ // BT         # batch tiles per core
    NB_SUB = BT // P        # 128-row subtiles per batch tile
    H = BT // 512           # moving-dim halves for matmul 1
    NGRP = NB_SUB // GRP

    nc = bacc.Bacc("TRN2", target_bir_lowering=False, debug=False,
                   num_devices=N_CORES)

    x_d = nc.dram_tensor("x16", [BS_ // BT, KT, BT, P], BF16,
                         kind="ExternalInput").ap()
    iw_d = nc.dram_tensor("iwp", [NDI, P, KT * P], BF16,
                          kind="ExternalInput").ap()
    ow_d = nc.dram_tensor("owp", [NDM, NDG, P, DIG * 512], BF16,
                          kind="ExternalInput").ap()
    c_d = nc.dram_tensor("cpb", [P, NDI], F32, kind="ExternalInput").ap()
    b_d = nc.dram_tensor("bpb", [P, NDI], F32, kind="ExternalInput").ap()
    ob_d = nc.dram_tensor("ob", [P, DM], F32, kind="ExternalInput").ap()
    out_d = nc.dram_tensor("out", [BS_, DM], F32, kind="ExternalOutput").ap()

    with tile.TileContext(nc) as tc:
        with (
            tc.tile_pool(name="const", bufs=1) as const,
            tc.tile_pool(name="xT", bufs=1) as xTp,
            tc.tile_pool(name="g", bufs=1) as gp,
            tc.tile_pool(name="iw", bufs=6) as iwp,
            tc.tile_pool(name="ow", bufs=12) as owp,
            tc.tile_pool(name="osb", bufs=2) as osbp,
            tc.tile_pool(name="trash", bufs=2) as trashp,
            tc.tile_pool(name="psZ", bufs=3, space="PSUM") as psZ,
            tc.tile_pool(name="psO", bufs=5, space="PSUM") as psO,
        ):
            c_sb = const.tile([P, NDI], F32)
            nc.sync.dma_start(c_sb[:], c_d)
            b_sb = const.tile([P, NDI], F32)
            nc.sync.dma_start(b_sb[:], b_d)
            ob_sb = const.tile([P, DM], F32)
            nc.sync.dma_start(ob_sb[:], ob_d)

            # one tile per k-slab so RAW/WAR deps are per-slab. Single
            # buffer per slab: the WAR on M1(t)'s reads is what defers
            # tile t+1's transposes into the M2(t) window — the scheduler
            # orders ready instructions greedily, so only data deps can
            # keep the scalar-queue transposes clear of the sync-queue
            # prologue (concurrent XBAR transposes on two queues corrupt).
            g = gp.tile([P, NDI, BT], BF16)
            xT = {}

            def emit_xT(t, kt, eng=None):
                """XBAR DMA-transpose one [BT, 128] slab of x into xT."""
                slab = xTp.tile([P, BT], BF16, tag=f"xT{kt}",
                                name=f"xT{kt}_{t}")
                xT[(t, kt)] = slab
                (eng or nc.scalar).dma_start(
                    slab[:], x_d[t, kt], transpose=True)

            def load_iw(di):
                if IW_TAG:
                    iw_t = iwp.tile([P, KT, P], BF16, tag="iw_t", name="iw_t")
                else:
                    iw_t = iwp.tile([P, KT, P], BF16, name="iw_t")
                nc.sync.dma_start(iw_t[:], iw_d[di].rearrange(
                    "p (kt m) -> p kt m", kt=KT))
                return iw_t

            def load_ow(dmc, dg):
                ow_t = owp.tile([P, DIG, 512], BF16, tag="ow_t", name="ow_t")
                nc.sync.dma_start(
                    ow_t[:], ow_d[dmc, dg].rearrange(
                        "p (s n) -> p s n", s=DIG))
                return ow_t

            iw_pf = {}
            for t in range(NBT):
                if t == 0:
                    # a couple of in_w chunks ahead of the transposes on the
                    # sync queue so M1 can start as soon as slab 0 lands
                    for d in range(2):
                        iw_pf[d] = load_iw(d)
                    # prologue transposes on sync: scalar queue must stay
                    # free for the first activations (psZ turnover)
                    for kt in range(KT):
                        emit_xT(0, kt, nc.sync)
                    # warm-up matmuls: keep the PE busy through the
                    # prologue so the HAM clock-gate reaches (and holds)
                    # 8/8 before the real stream begins
                    dw = const.tile([P, P], BF16)
                    nc.vector.memset(dw[:], 0.0)
                    dm = const.tile([P, 512], BF16)
                    nc.vector.memset(dm[:], 0.0)
                    for w in range(WARMUP_MMS):
                        dps = psZ.tile([P, 512], F32, tag="zp",
                                       name=f"warm_{w}")
                        nc.tensor.matmul(dps[:], dw[:], dm[:],
                                         start=True, stop=True)

                # ---- phase M1: z^T = in_w^T @ x^T ; g = silu(z*c + b) ----
                for di in range(NDI):
                    iw_t = iw_pf.pop(di, None)
                    if iw_t is None:
                        iw_t = load_iw(di)
                    zps = [psZ.tile([P, 512], F32, tag="zp", name=f"zp_{h}")
                           for h in range(H)]
                    for kt in range(KT):
                        for h in range(H):
                            nc.tensor.matmul(
                                zps[h][:],
                                iw_t[:, kt, :],
                                xT[(t, kt)][:, h * 512:(h + 1) * 512],
                                start=(kt == 0), stop=(kt == KT - 1))
                    for h in range(H):
                        nc.scalar.activation(
                            g[:, di, h * 512:(h + 1) * 512], zps[h][:], SILU,
                            bias=b_sb[:, di:di + 1], scale=c_sb[:, di:di + 1])
                # ---- phase M2: out = g^T @ out_w + out_b ----
                ui = 0
                for dmc in range(NDM):
                    for grp in range(NGRP):
                        ops = [psO.tile([P, 512], F32, tag="ps_o",
                                        name=f"ops_{j}")
                               for j in range(GRP)]
                        for dg in range(NDG):
                            ow_t = load_ow(dmc, dg)
                            for s in range(DIG):
                                di = dg * DIG + s
                                for j in range(GRP):
                                    bs = grp * GRP + j
                                    nc.tensor.matmul(
                                        ops[j][:],
                                        g[:, di, bs * P:(bs + 1) * P],
                                        ow_t[:, s, :],
                                        start=(di == 0),
                                        stop=(di == NDI - 1))
                        osb = osbp.tile([P, GRP, 512], F32)
                        for j in range(GRP):
                            nc.vector.tensor_tensor(
                                osb[:, j, :], ops[j][:],
                                ob_sb[:, dmc * 512:(dmc + 1) * 512],
                                mybir.AluOpType.add)
                        r0 = t * BT + grp * GRP * P
                        nc.gpsimd.dma_start(
                            out_d[r0:r0 + GRP * P,
                                  dmc * 512:(dmc + 1) * 512].rearrange(
                                      "(s p) n -> p s n", p=P),
                            osb[:])
                        if t + 1 < NBT:
                            for _ in range(2):
                                if ui < KT:
                                    # pace the transpose into THIS psum
                                    # group's window: a tiny DVE read of the
                                    # old slab, gated on this group's osb,
                                    # becomes the slab's last reader, so the
                                    # transpose's WAR defers it here instead
                                    # of letting all 16 burst at M2 start
                                    tr = trashp.tile([1, 4], F32, tag="tr",
                                                     name="tr")
                                    nc.vector.tensor_tensor(
                                        tr[:], xT[(t, ui)][0:1, 0:4],
                                        osb[0:1, 0, 0:4],
                                        mybir.AluOpType.add)
                                    emit_xT(t + 1, ui)
                                    ui += 1
                            # prefetch next tile's first in_w chunks in the
                            # second half of M2, after the transpose burst
                            pf = dmc * NGRP + grp
                            if 4 <= pf < 8 and PREFETCH_IW:
                                iw_pf[pf - 4] = load_iw(pf - 4)
    nc.compile()
    return nc


_NC_CACHE = {}


def _get_nc(key):
    if key not in _NC_CACHE:
        cfg = dict(BT=key[0])
        _NC_CACHE[key] = build_nc(cfg)
    return _NC_CACHE[key]


CONFIG = (1024,)


def _softplus(v):
    return np.logaddexp(0.0, v)


def prep_inputs(x, in_w, in_b, conv_w, conv_b, A_log, B, C, Dp, dt,
                out_w, out_b):
    """Host-side prep shared by kernel() and the test harness."""
    import ml_dtypes
    bf16 = ml_dtypes.bfloat16

    bt = CONFIG[0]
    x16 = np.asarray(x, np.float32).astype(bf16)
    # contiguous [BT, 128] slabs so each XBAR DMA-transpose reads DRAM
    # sequentially: [BS, DM] -> [NBT, KT, BT, P]
    x16 = x16.reshape(N_CORES, BS // bt, bt, KT, P).transpose(0, 1, 3, 2, 4)

    # in_w [DM, DI] -> iwp [NDI, P(dm-in-chunk), KT*P(di-in-chunk)]
    iw = np.asarray(in_w, np.float32).astype(bf16)
    iwp = np.ascontiguousarray(
        iw.reshape(KT, P, NDI, P).transpose(2, 1, 0, 3).reshape(
            NDI, P, KT * P))

    # out_w [DI, DM] -> owp [NDM, NDG, P(di-in-chunk), DIG*512]
    ow = np.asarray(out_w, np.float32).astype(bf16)
    owp = np.ascontiguousarray(
        ow.reshape(NDG, DIG, P, NDM, 512).transpose(3, 0, 2, 1, 4).reshape(
            NDM, NDG, P, DIG * 512))

    # host precompute of the per-channel SSM/conv collapse
    c = (np.asarray(conv_w, np.float32)[:, -1]
         + _softplus(np.asarray(dt, np.float32))
         * np.sum(np.asarray(B, np.float32) * np.asarray(C, np.float32), -1)
         + np.asarray(Dp, np.float32))
    b_eff = np.asarray(in_b, np.float32) * c + np.asarray(conv_b, np.float32)

    c_pb = np.ascontiguousarray(c.reshape(NDI, P).T)
    b_pb = np.ascontiguousarray(b_eff.reshape(NDI, P).T)
    ob_rep = np.ascontiguousarray(
        np.broadcast_to(np.asarray(out_b, np.float32), (P, DM)))

    in_maps = []
    for i in range(N_CORES):
        in_maps.append({
            "x16": np.ascontiguousarray(x16[i]),
            "iwp": iwp,
            "owp": owp,
            "cpb": c_pb,
            "bpb": b_pb,
            "ob": ob_rep,
        })
    return in_maps


def kernel(x, in_w, in_b, conv_w, conv_b, A_log, B, C, Dp, dt, out_w, out_b):
    in_maps = prep_inputs(x, in_w, in_b, conv_w, conv_b, A_log, B, C, Dp,
                          dt, out_w, out_b)
    nc = _get_nc(CONFIG)
    out = np.empty((B_FULL, DM), dtype=np.float32)
    try:
        res = run_bass_kernel_spmd(nc, in_maps, core_ids=list(range(N_CORES)))
        for i in range(N_CORES):
            out[i * BS:(i + 1) * BS] = res.results[i]["out"]
    except Exception:
        # The accelerator occasionally hits a transient unrecoverable fault
        # that poisons this process's PJRT client; a fresh process recovers.
        # Retry the device execution in a subprocess.
        _run_in_subprocess(in_maps, out)
    return out


def _run_in_subprocess(in_maps, out):
    import pickle
    import subprocess
    import sys
    import tempfile

    with tempfile.TemporaryDirectory() as td:
        in_path = f"{td}/in.pkl"
        out_path = f"{td}/out.npy"
        with open(in_path, "wb") as f:
            pickle.dump({"config": CONFIG, "in_maps": in_maps}, f,
                        protocol=pickle.HIGHEST_PROTOCOL)
        for attempt in range(3):
            r = subprocess.run(
                [sys.executable, __file__, "--worker", in_path, out_path],
                capture_output=True)
            if r.returncode == 0:
                break
            if attempt == 2:
                raise RuntimeError(
                    f"device worker failed 3x: {r.stderr[-2000:]!r}")
        out[:] = np.load(out_path)


def _worker_main(in_path, out_path):
    import pickle
    with open(in_path, "rb") as f:
        job = pickle.load(f)
    nc = _get_nc(tuple(job["config"]))
    res = run_bass_kernel_spmd(nc, job["in_maps"],
                               core_ids=list(range(N_CORES)))
    out = np.empty((B_FULL, DM), dtype=np.float32)
    for i in range(N_CORES):
        out[i * BS:(i + 1) * BS] = res.results[i]["out"]
    np.save(out_path, out)


if __name__ == "__main__":
    import sys as _sys
    if len(_sys.argv) == 4 and _sys.argv[1] == "--worker":
        _worker_main(_sys.argv[2], _sys.argv[3])


# revision 43
# speedup vs baseline: 1.0446x; 1.0081x over previous
"""Trainium2 Bass kernel for a dense (length-1 sequence) Mamba block.

The reference computation reduces algebraically to:
    z   = x @ in_w                                  # (B, d_inner)
    g   = silu(z * c + b_eff)                       # per-channel scale/bias
    out = g @ out_w + out_b                         # (B, d_model)
with
    c     = conv_w[:, -1] + softplus(dt) * sum(B*C, -1) + Dp
    b_eff = (in_b * c) + conv_b
(c, b_eff are tiny per-channel vectors, computed on host.)

Strategy: data-parallel over 8 NeuronCores (batch 32768 -> 8 x 4096).
All-bf16 datapath (inputs converted on host; well within tolerance).
Per core, batch is processed in tiles of BT rows:
  x^T tiles are produced by the XBAR DMA-transpose (HBM -> SBUF) so the
  PE array spends zero cycles on transposes.
  phase M1: z^T[di, b] accumulated over d_model; Silu fused on ScalarE
            with per-partition scale/bias -> g [di, b] (bf16)
  phase M2: out[b, dm] accumulated over d_inner with g slices as the
            stationary operand and out_w tiles moving (natural output
            layout); out_b added on drain.
The kernel is PE-bound (2 x 4096x2048x4096 MACs/core); everything else
overlaps under the matmul stream.
"""

import numpy as np

import concourse.bass as bass
import concourse.tile as tile
from concourse import bacc, mybir
from concourse.bass_utils import run_bass_kernel_spmd

P = 128
B_FULL = 32768
DM = 2048
DI = 4096
N_CORES = 8
BS = B_FULL // N_CORES  # rows per core

F32 = mybir.dt.float32
BF16 = mybir.dt.bfloat16
SILU = mybir.ActivationFunctionType.Silu

import os as _os
PREFETCH_IW = _os.environ.get("K_PREFETCH", "1") == "1"
# NOTE: concurrent XBAR DMA-transposes on both HWDGE queues corrupt data
# (shared xbar state) — keep all transposes on the scalar queue.
SPLIT_PROLOGUE = _os.environ.get("K_SPLIT_PRO", "0") == "1"
IW_TAG = _os.environ.get("K_IW_TAG", "1") == "1"
WARMUP_MMS = int(_os.environ.get("K_WARMUP", "48"))

KT = DM // P            # 16 k-chunks for matmul 1
NDI = DI // P           # 32 d_inner chunks of 128
NDM = DM // 512         # 4 d_model chunks of 512
DIG = 4                 # d_inner chunks per out_w DMA batch
NDG = NDI // DIG        # 8 out_w loads per (dmc, grp)
GRP = 4                 # psum banks used by M2 accumulation


def build_nc(cfg):
    """Build the per-core Bass module. cfg: dict(BT=...)"""
    BT = cfg["BT"]
    BS_ = cfg.get("BS", BS)
    NBT = BS_ // BT         # batch tiles per core
    NB_SUB = BT // P        # 128-row subtiles per batch tile
    H = BT // 512           # moving-dim halves for matmul 1
    NGRP = NB_SUB // GRP

    nc = bacc.Bacc("TRN2", target_bir_lowering=False, debug=False,
                   num_devices=N_CORES)

    x_d = nc.dram_tensor("x16", [BS_ // BT, KT, BT, P], BF16,
                         kind="ExternalInput").ap()
    iw_d = nc.dram_tensor("iwp", [NDI, P, KT * P], BF16,
                          kind="ExternalInput").ap()
    ow_d = nc.dram_tensor("owp", [NDM, NDG, P, DIG * 512], BF16,
                          kind="ExternalInput").ap()
    c_d = nc.dram_tensor("cpb", [P, NDI], F32, kind="ExternalInput").ap()
    b_d = nc.dram_tensor("bpb", [P, NDI], F32, kind="ExternalInput").ap()
    ob_d = nc.dram_tensor("ob", [P, DM], F32, kind="ExternalInput").ap()
    out_d = nc.dram_tensor("out", [BS_, DM], F32, kind="ExternalOutput").ap()

    with tile.TileContext(nc) as tc:
        with (
            tc.tile_pool(name="const", bufs=1) as const,
            tc.tile_pool(name="xT", bufs=1) as xTp,
            tc.tile_pool(name="g", bufs=1) as gp,
            tc.tile_pool(name="iw", bufs=6) as iwp,
            tc.tile_pool(name="ow", bufs=12) as owp,
            tc.tile_pool(name="osb", bufs=2) as osbp,
            tc.tile_pool(name="trash", bufs=2) as trashp,
            tc.tile_pool(name="psZ", bufs=3, space="PSUM") as psZ,
            tc.tile_pool(name="psO", bufs=5, space="PSUM") as psO,
        ):
            c_sb = const.tile([P, NDI], F32)
            nc.sync.dma_start(c_sb[:], c_d)
            b_sb = const.tile([P, NDI], F32)
            nc.sync.dma_start(b_sb[:], b_d)
            ob_sb = const.tile([P, DM], F32)
            nc.sync.dma_start(ob_sb[:], ob_d)

            # one tile per k-slab so RAW/WAR deps are per-slab. Single
            # buffer per slab: the WAR on M1(t)'s reads is what defers
            # tile t+1's transposes into the M2(t) window — the scheduler
            # orders ready instructions greedily, so only data deps can
            # keep the scalar-queue transposes clear of the sync-queue
            # prologue (concurrent XBAR transposes on two queues corrupt).
            g = gp.tile([P, NDI, BT], BF16)
            xT = {}

            def emit_xT(t, kt, eng=None):
                """XBAR DMA-transpose one [BT, 128] slab of x into xT."""
                slab = xTp.tile([P, BT], BF16, tag=f"xT{kt}",
                                name=f"xT{kt}_{t}")
                xT[(t, kt)] = slab
                (eng or nc.scalar).dma_start(
                    slab[:], x_d[t, kt], transpose=True)

            def load_iw(di):
                if IW_TAG:
                    iw_t = iwp.tile([P, KT, P], BF16, tag="iw_t", name="iw_t")
                else:
                    iw_t = iwp.tile([P, KT, P], BF16, name="iw_t")
                nc.sync.dma_start(iw_t[:], iw_d[di].rearrange(
                    "p (kt m) -> p kt m", kt=KT))
                return iw_t

            def load_ow(dmc, dg):
                ow_t = owp.tile([P, DIG, 512], BF16, tag="ow_t", name="ow_t")
                nc.sync.dma_start(
                    ow_t[:], ow_d[dmc, dg].rearrange(
                        "p (s n) -> p s n", s=DIG))
                return ow_t

            iw_pf = {}
            for t in range(NBT):
                if t == 0:
                    # a few in_w chunks ahead of the transposes on the
                    # sync queue so M1 never drips on early iw supply
                    for d in range(4):
                        iw_pf[d] = load_iw(d)
                    # prologue transposes on sync: scalar queue must stay
                    # free for the first activations (psZ turnover)
                    for kt in range(KT):
                        emit_xT(0, kt, nc.sync)
                    # warm-up matmuls: keep the PE busy through the
                    # prologue so the HAM clock-gate reaches (and holds)
                    # 8/8 before the real stream begins
                    dw = const.tile([P, P], BF16)
                    nc.vector.memset(dw[:], 0.0)
                    dm = const.tile([P, 512], BF16)
                    nc.vector.memset(dm[:], 0.0)
                    for w in range(WARMUP_MMS):
                        dps = psZ.tile([P, 512], F32, tag="zp",
                                       name=f"warm_{w}")
                        nc.tensor.matmul(dps[:], dw[:], dm[:],
                                         start=True, stop=True)

                # ---- phase M1: z^T = in_w^T @ x^T ; g = silu(z*c + b) ----
                for di in range(NDI):
                    iw_t = iw_pf.pop(di, None)
                    if iw_t is None:
                        iw_t = load_iw(di)
                    zps = [psZ.tile([P, 512], F32, tag="zp", name=f"zp_{h}")
                           for h in range(H)]
                    for kt in range(KT):
                        for h in range(H):
                            nc.tensor.matmul(
                                zps[h][:],
                                iw_t[:, kt, :],
                                xT[(t, kt)][:, h * 512:(h + 1) * 512],
                                start=(kt == 0), stop=(kt == KT - 1))
                    for h in range(H):
                        nc.scalar.activation(
                            g[:, di, h * 512:(h + 1) * 512], zps[h][:], SILU,
                            bias=b_sb[:, di:di + 1], scale=c_sb[:, di:di + 1])
                # ---- phase M2: out = g^T @ out_w + out_b ----
                ui = 0
                for dmc in range(NDM):
                    for grp in range(NGRP):
                        ops = [psO.tile([P, 512], F32, tag="ps_o",
                                        name=f"ops_{j}")
                               for j in range(GRP)]
                        for dg in range(NDG):
                            ow_t = load_ow(dmc, dg)
                            for s in range(DIG):
                                di = dg * DIG + s
                                for j in range(GRP):
                                    bs = grp * GRP + j
                                    nc.tensor.matmul(
                                        ops[j][:],
                                        g[:, di, bs * P:(bs + 1) * P],
                                        ow_t[:, s, :],
                                        start=(di == 0),
                                        stop=(di == NDI - 1))
                        osb = osbp.tile([P, GRP, 512], F32)
                        for j in range(GRP):
                            nc.vector.tensor_tensor(
                                osb[:, j, :], ops[j][:],
                                ob_sb[:, dmc * 512:(dmc + 1) * 512],
                                mybir.AluOpType.add)
                        r0 = t * BT + grp * GRP * P
                        nc.gpsimd.dma_start(
                            out_d[r0:r0 + GRP * P,
                                  dmc * 512:(dmc + 1) * 512].rearrange(
                                      "(s p) n -> p s n", p=P),
                            osb[:])
                        if t + 1 < NBT:
                            pg = dmc * NGRP + grp
                            for _ in range((3, 3, 3, 3, 2, 2, 0, 0)[pg]):
                                if ui < KT:
                                    # pace the transpose into THIS psum
                                    # group's window: a tiny DVE read of the
                                    # old slab, gated on this group's osb,
                                    # becomes the slab's last reader, so the
                                    # transpose's WAR defers it here instead
                                    # of letting all 16 burst at M2 start
                                    tr = trashp.tile([1, 4], F32, tag="tr",
                                                     name="tr")
                                    nc.vector.tensor_tensor(
                                        tr[:], xT[(t, ui)][0:1, 0:4],
                                        osb[0:1, 0, 0:4],
                                        mybir.AluOpType.add)
                                    emit_xT(t + 1, ui)
                                    ui += 1
                            # prefetch next tile's first in_w chunks in the
                            # second half of M2, after the transpose burst
                            pf = dmc * NGRP + grp
                            if 4 <= pf < 8 and PREFETCH_IW:
                                iw_pf[pf - 4] = load_iw(pf - 4)
    nc.compile()
    return nc


_NC_CACHE = {}


def _get_nc(key):
    if key not in _NC_CACHE:
        cfg = dict(BT=key[0])
        _NC_CACHE[key] = build_nc(cfg)
    return _NC_CACHE[key]


CONFIG = (1024,)


def _softplus(v):
    return np.logaddexp(0.0, v)


def prep_inputs(x, in_w, in_b, conv_w, conv_b, A_log, B, C, Dp, dt,
                out_w, out_b):
    """Host-side prep shared by kernel() and the test harness."""
    import ml_dtypes
    bf16 = ml_dtypes.bfloat16

    bt = CONFIG[0]
    x16 = np.asarray(x, np.float32).astype(bf16)
    # contiguous [BT, 128] slabs so each XBAR DMA-transpose reads DRAM
    # sequentially: [BS, DM] -> [NBT, KT, BT, P]
    x16 = x16.reshape(N_CORES, BS // bt, bt, KT, P).transpose(0, 1, 3, 2, 4)

    # in_w [DM, DI] -> iwp [NDI, P(dm-in-chunk), KT*P(di-in-chunk)]
    iw = np.asarray(in_w, np.float32).astype(bf16)
    iwp = np.ascontiguousarray(
        iw.reshape(KT, P, NDI, P).transpose(2, 1, 0, 3).reshape(
            NDI, P, KT * P))

    # out_w [DI, DM] -> owp [NDM, NDG, P(di-in-chunk), DIG*512]
    ow = np.asarray(out_w, np.float32).astype(bf16)
    owp = np.ascontiguousarray(
        ow.reshape(NDG, DIG, P, NDM, 512).transpose(3, 0, 2, 1, 4).reshape(
            NDM, NDG, P, DIG * 512))

    # host precompute of the per-channel SSM/conv collapse
    c = (np.asarray(conv_w, np.float32)[:, -1]
         + _softplus(np.asarray(dt, np.float32))
         * np.sum(np.asarray(B, np.float32) * np.asarray(C, np.float32), -1)
         + np.asarray(Dp, np.float32))
    b_eff = np.asarray(in_b, np.float32) * c + np.asarray(conv_b, np.float32)

    c_pb = np.ascontiguousarray(c.reshape(NDI, P).T)
    b_pb = np.ascontiguousarray(b_eff.reshape(NDI, P).T)
    ob_rep = np.ascontiguousarray(
        np.broadcast_to(np.asarray(out_b, np.float32), (P, DM)))

    in_maps = []
    for i in range(N_CORES):
        in_maps.append({
            "x16": np.ascontiguousarray(x16[i]),
            "iwp": iwp,
            "owp": owp,
            "cpb": c_pb,
            "bpb": b_pb,
            "ob": ob_rep,
        })
    return in_maps


def kernel(x, in_w, in_b, conv_w, conv_b, A_log, B, C, Dp, dt, out_w, out_b):
    in_maps = prep_inputs(x, in_w, in_b, conv_w, conv_b, A_log, B, C, Dp,
                          dt, out_w, out_b)
    nc = _get_nc(CONFIG)
    out = np.empty((B_FULL, DM), dtype=np.float32)
    try:
        res = run_bass_kernel_spmd(nc, in_maps, core_ids=list(range(N_CORES)))
        for i in range(N_CORES):
            out[i * BS:(i + 1) * BS] = res.results[i]["out"]
    except Exception:
        # The accelerator occasionally hits a transient unrecoverable fault
        # that poisons this process's PJRT client; a fresh process recovers.
        # Retry the device execution in a subprocess.
        _run_in_subprocess(in_maps, out)
    return out


def _run_in_subprocess(in_maps, out):
    import pickle
    import subprocess
    import sys
    import tempfile

    with tempfile.TemporaryDirectory() as td:
        in_path = f"{td}/in.pkl"
        out_path = f"{td}/out.npy"
        with open(in_path, "wb") as f:
            pickle.dump({"config": CONFIG, "in_maps": in_maps}, f,
                        protocol=pickle.HIGHEST_PROTOCOL)
        for attempt in range(3):
            r = subprocess.run(
                [sys.executable, __file__, "--worker", in_path, out_path],
                capture_output=True)
            if r.returncode == 0:
                break
            if attempt == 2:
                raise RuntimeError(
                    f"device worker failed 3x: {r.stderr[-2000:]!r}")
        out[:] = np.load(out_path)


def _worker_main(in_path, out_path):
    import pickle
    with open(in_path, "rb") as f:
        job = pickle.load(f)
    nc = _get_nc(tuple(job["config"]))
    res = run_bass_kernel_spmd(nc, job["in_maps"],
                               core_ids=list(range(N_CORES)))
    out = np.empty((B_FULL, DM), dtype=np.float32)
    for i in range(N_CORES):
        out[i * BS:(i + 1) * BS] = res.results[i]["out"]
    np.save(out_path, out)


if __name__ == "__main__":
    import sys as _sys
    if len(_sys.argv) == 4 and _sys.argv[1] == "--worker":
        _worker_main(_sys.argv[2], _sys.argv[3])
